# revision 26
# baseline (speedup 1.0000x reference)
"""Distributed Bass kernel for pre-LN multi-head attention on 8 TRN2 NeuronCores.

Problem: x[2, 2048, 1024] -> LayerNorm -> QKV (16 heads x 64) -> softmax(QK^T/8)V
         -> out proj [1024] + bias.

Sharding (v2): core = (batch b, head group hg) — data parallel over B, tensor
parallel over heads (4 heads/core). Each core projects Q/K/V only for its own
4 heads over the full 2048 tokens (no redundant K/V recompute), runs attention,
and emits a PARTIAL output projection [2048, 1024] (f32). The host sums the 4
partials per batch and adds b_out — the same class of work as gather/unshard.

Measured-HW-calibrated choices (see probes.py):
- 64-contraction score matmuls run ~3x slower per row than 128-contraction on
  real HW, so K is stored zero-padded to 128 rows per head (parity trick: even
  heads real in rows 0-63, odd heads in 64-127; the zero rows null out the
  other head's Q in the full-128-partition moving operand).
- PE transposes are ~4x the model cost; attention output is produced directly
  transposed ([dh, q] orientation) so none are needed.
- exp runs ~2x faster than the model (≈0.46 ns/elem) — the Act engine only
  does exp (LN stats are ones-matmuls on PE). The softmax denominator rides
  the av matmul as a 65th ones-column of V (separate [128,1]-stationary den
  matmuls measured 2.2x slower for the whole attention loop — they thrash the
  PE weight pipeline). All heads' av lands at partitions 0-64; odd heads are
  staged and partition-shifted to rows 64-127 with one SBUF->SBUF DMA per
  query block (DVE cannot write across partitions).
- LayerNorm is folded into the projections as a rank-1 correction:
  proj[col,t] = a[t]*(W^T x)[col,t] + c[t]*colsum(W)[col], a=rstd, c=-mean*rstd
  (ln_scale folded into W host-side; ln_bias@W == 0 for this model). This lets
  raw QKV matmuls start before LN stats finish.

All matmul operands bf16 (fp8 fails the 2e-2 max-rel-err gate — measured),
accumulation f32.
"""

import numpy as np
import ml_dtypes

import concourse.bass as bass
import concourse.mybir as mybir
import concourse.tile as tile
from concourse import bacc
from concourse.bass import ts, ds
from concourse.bass_utils import run_bass_kernel_spmd

B, S, D = 2, 2048, 1024
H, DH = 16, 64
INNER = H * DH
N_CORES = 8
H_PC = 4               # heads per core
NDC = 8                # 128-row contraction chunks over D
NTB = 4                # token blocks of 512
NKC = 16               # kpos chunks of 128
NTT = 16               # token tiles of 128
F32 = mybir.dt.float32
BF16 = mybir.dt.bfloat16
AF = mybir.ActivationFunctionType
OP = mybir.AluOpType


DEBUG = False


def _build_iter(nc, tc, ext, it, rep_proj=1, rep_attn=1):
    (xT_ext, wq_ext, wsum_ext, wo_ext, out_ext) = ext
    dbg = {}
    if DEBUG and it == 0:
        for nm, shp, dt in [("d_qk", [128, 4, S], BF16),
                            ("d_kpad", [128, H_PC, S], BF16),
                            ("d_v", [128, NKC, H_PC * 65], BF16),
                            ("d_attnT", [128, 2, S], BF16),
                            ("d_abc", [128, S], BF16),
                            ("d_cbc", [128, S], BF16),
                            ("d_atok", [128, NKC], F32),
                            ("d_ex00", [128, 2, 512], BF16),
                            ("d_den00", [1, 512], F32),
                            ("d_rbc00", [128, 512], F32)]:
            dbg[nm] = nc.declare_dram_parameter(nm, shp, dt, isOutput=True)

    with tc.tile_pool(name=f"const{it}", bufs=1) as constp, \
         tc.tile_pool(name=f"pers{it}", bufs=1) as pers, \
         tc.tile_pool(name=f"dram{it}", bufs=1, space="DRAM") as dram:

        # ---- constants ----
        ones_st = constp.tile([128, 1], BF16)
        nc.vector.memset(ones_st[:], 1.0)
        eps_t = constp.tile([1, 1], F32)
        nc.vector.memset(eps_t[:], 1e-6)
        wsum_t = constp.tile([128, 6], F32)       # per-partition col sums
        nc.sync.dma_start(wsum_t[:], wsum_ext[:].rearrange("(c p) -> p c", p=128))
        vwsum_bc = constp.tile([128, 256], F32)   # v col sums, bcast across parts
        nc.sync.dma_start(
            vwsum_bc[:],
            wsum_ext[ds(512, 256)].rearrange(
                "(o d) -> o d", o=1)[0:1, :].to_broadcast((128, 256)))

        # ---- persistent activations ----
        xT = pers.tile([128, NDC, S], BF16)       # raw x^T (d = c*128+p)
        qk_sb = pers.tile([128, 4, S], BF16)      # q (cc 0-1), k packed (cc 2-3)
        k_pad = pers.tile([128, H_PC, S], BF16)   # per-head K, parity-zero-padded
        v_sb = pers.tile([128, NKC, H_PC * 65], BF16)  # V + ones col per head
        stg = pers.tile([64, 2, S], BF16)         # odd-head attn staging
        attn_nT = pers.tile([128, 2, S], BF16)    # normalized attn out, [inner, q]
        a_bc = pers.tile([128, S], BF16)          # rstd, bcast across partitions
        c_bc = pers.tile([128, S], BF16)          # -mean*rstd, bcast
        a_tok = pers.tile([128, NKC], F32)        # rstd, tokens on partitions
        c_tok = pers.tile([128, NKC], F32)

        a_dram = dram.tile([S], BF16)
        c_dram = dram.tile([S], BF16)
        a32_dram = dram.tile([S], F32)
        c32_dram = dram.tile([S], F32)

        wq = pers.tile([128, NDC, 768], BF16)
        wo = pers.tile([128, 2, D], BF16)

        # ones cols of v_sb (softmax denominator rides the av matmul)
        nc.gpsimd.memset(
            v_sb[:].rearrange("p k (h c) -> p k h c", c=65)[:, :, :, 64:65], 1.0)
        # zero halves of k_pad (parity trick)
        for h in range(H_PC):
            nc.gpsimd.memset(k_pad[ds((1 - h % 2) * 64, 64), h, :], 0.0)

        # ---- loads (chunked so LN stats can start early) ----
        for tb in range(NTB):
            nc.sync.dma_start(
                xT[:, :, ds(tb * 512, 512)],
                xT_ext[:, ds(tb * 512, 512)].rearrange("(c p) t -> p c t", p=128))
        nc.sync.dma_start(wq[:], wq_ext[:, :].rearrange("(c p) n -> p c n", p=128))
        nc.sync.dma_start(wo[:], wo_ext[:, :].rearrange("(c p) n -> p c n", p=128))

        # =============== LN stats (PE ones-matmuls) -> a, c ===============
        with tc.tile_pool(name=f"st{it}", bufs=2) as stp, \
             tc.tile_pool(name=f"stps{it}", bufs=2, space="PSUM") as stps:
            sq = pers.tile([128, NDC, S], BF16)
            for tb in range(NTB):
                sl = ds(tb * 512, 512)
                nc.vector.tensor_tensor(
                    sq[:, :, sl], xT[:, :, sl], xT[:, :, sl], op=OP.mult)
                s_ps = stps.tile([1, 512], F32, tag="s")
                q_ps = stps.tile([1, 512], F32, tag="q")
                for dc in range(NDC):
                    nc.tensor.matmul(s_ps[:], ones_st[:], xT[:, dc, sl],
                                     start=(dc == 0), stop=(dc == NDC - 1))
                for dc in range(NDC):
                    nc.tensor.matmul(q_ps[:], ones_st[:], sq[:, dc, sl],
                                     start=(dc == 0), stop=(dc == NDC - 1))
                mean = stp.tile([1, 512], F32, tag="mean")
                nc.vector.tensor_scalar(mean[:], s_ps[:], 1.0 / D, None, op0=OP.mult)
                msq = stp.tile([1, 512], F32, tag="msq")
                nc.vector.tensor_tensor(msq[:], mean[:], mean[:], op=OP.mult)
                var = stp.tile([1, 512], F32, tag="var")
                nc.vector.scalar_tensor_tensor(
                    var[:], q_ps[:], 1.0 / D, msq[:], op0=OP.mult, op1=OP.subtract)
                std = stp.tile([1, 512], F32, tag="std")
                nc.scalar.activation(std[:], var[:], AF.Sqrt, bias=eps_t[:])
                rstd = stp.tile([1, 512], F32, tag="rstd")
                nc.vector.reciprocal(rstd[:], std[:])
                c_f = stp.tile([1, 512], F32, tag="c_f")
                nc.vector.scalar_tensor_tensor(
                    c_f[:], mean[:], -1.0, rstd[:], op0=OP.mult, op1=OP.mult)
                a_bf = stp.tile([1, 512], BF16, tag="a_bf")
                nc.vector.tensor_scalar(a_bf[:], rstd[:], 1.0, None, op0=OP.mult)
                c_bf = stp.tile([1, 512], BF16, tag="c_bf")
                nc.vector.tensor_scalar(c_bf[:], c_f[:], 1.0, None, op0=OP.mult)
                sl1 = ds(tb * 512, 512)
                nc.sync.dma_start(
                    a_dram[sl1].rearrange("(o t) -> o t", o=1), a_bf[:])
                nc.sync.dma_start(
                    c_dram[sl1].rearrange("(o t) -> o t", o=1), c_bf[:])
                nc.sync.dma_start(
                    a32_dram[sl1].rearrange("(o t) -> o t", o=1), rstd[:])
                nc.sync.dma_start(
                    c32_dram[sl1].rearrange("(o t) -> o t", o=1), c_f[:])
            nc.sync.dma_start(
                a_bc[:],
                a_dram[:].rearrange("(o t) -> o t", o=1)[0:1, :].to_broadcast((128, S)))
            nc.sync.dma_start(
                c_bc[:],
                c_dram[:].rearrange("(o t) -> o t", o=1)[0:1, :].to_broadcast((128, S)))
            nc.sync.dma_start(a_tok[:], a32_dram[:].rearrange("(k p) -> p k", p=128))
            nc.sync.dma_start(c_tok[:], c32_dram[:].rearrange("(k p) -> p k", p=128))
            # fold a into the activations (over sq's space; stats are done
            # with it): xa = x * a  =>  proj needs only the +c*wsum fixup
            xa = sq
            for tb in range(NTB):
                sl = ds(tb * 512, 512)
                a_b3 = a_bc[:, sl].rearrange(
                    "p (o t) -> p o t", o=1).to_broadcast((128, NDC, 512))
                nc.vector.tensor_tensor(
                    xa[:, :, sl], xT[:, :, sl], a_b3, op=OP.mult)

        # =============== QKV projections (raw x; LN as rank-1 fixup) =========
        for rp in range(rep_proj):
          with tc.tile_pool(name=f"qv{it}_{rp}", bufs=3) as qvp, \
             tc.tile_pool(name=f"qkps{it}_{rp}", bufs=3, space="PSUM") as qkps:
            # K first so scores can start earliest; cc: 0-1 q, 2-3 k
            for cc in (2, 3, 0, 1):
                for tb in range(NTB):
                    sl = ds(tb * 512, 512)
                    ps = qkps.tile([128, 512], F32, tag="qk")
                    for dc in range(NDC):
                        nc.tensor.matmul(ps[:], wq[:, dc, ts(cc, 128)],
                                         xa[:, dc, sl],
                                         start=(dc == 0), stop=(dc == NDC - 1))
                    if cc < 2:
                        nc.vector.scalar_tensor_tensor(
                            qk_sb[:, cc, sl], c_bc[:, sl], wsum_t[:, cc:cc + 1],
                            ps[:], op0=OP.mult, op1=OP.add)
                    else:
                        for par in range(2):       # head = (cc-2)*2 + par
                            h = (cc - 2) * 2 + par
                            pr = ds(par * 64, 64)
                            nc.vector.scalar_tensor_tensor(
                                k_pad[pr, h, sl], c_bc[pr, sl],
                                wsum_t[pr, cc:cc + 1], ps[pr, :],
                                op0=OP.mult, op1=OP.add)
            # V in [token, vcol] orientation
            for kc in range(NKC):
                ps = qkps.tile([128, 256], F32, tag="v")
                for dc in range(NDC):
                    nc.tensor.matmul(ps[:], xa[:, dc, ds(kc * 128, 128)],
                                     wq[:, dc, ds(512, 256)],
                                     start=(dc == 0), stop=(dc == NDC - 1))
                nc.vector.scalar_tensor_tensor(
                    v_sb[:, kc, :].rearrange("p (h c) -> p h c", c=65)[:, :, 0:64],
                    vwsum_bc[:].rearrange("p (h c) -> p h c", c=64),
                    c_tok[:, kc:kc + 1],
                    ps[:].rearrange("p (h c) -> p h c", c=64),
                    op0=OP.mult, op1=OP.add)

        # =============== attention + output projection ===============
        for ra in range(rep_attn):
          with tc.tile_pool(name=f"att{it}_{ra}", bufs=5) as attp, \
             tc.tile_pool(name=f"nrm{it}_{ra}", bufs=4) as nrmp, \
             tc.tile_pool(name=f"out{it}_{ra}", bufs=2) as outp, \
             tc.tile_pool(name=f"scps{it}_{ra}", bufs=3, space="PSUM") as scps, \
             tc.tile_pool(name=f"avps{it}_{ra}", bufs=2, space="PSUM") as avps:
            for qb in range(NTB):
                qsl = ds(qb * 512, 512)
                for h in range(H_PC):
                    hc = h // 2
                    q_mv = qk_sb[:, hc, qsl]
                    av = avps.tile([128, 512], F32, tag="av",
                                   name=f"av{it}_{ra}_{qb}_{h}")
                    for kcp in range(8):
                        sc = scps.tile([128, 2, 512], F32, tag="sc",
                                       name=f"sc{it}_{ra}_{qb}_{h}_{kcp}")
                        for i in range(2):
                            kc = kcp * 2 + i
                            nc.tensor.matmul(
                                sc[:, i, :], k_pad[:, h, ds(kc * 128, 128)], q_mv,
                                start=True, stop=True)
                        ex = attp.tile([128, 2, 512], BF16, tag="ex")
                        nc.scalar.activation(ex[:], sc[:], AF.Exp, scale=0.125)
                        if dbg and qb == 0 and h == 0 and kcp == 0:
                            nc.sync.dma_start(dbg["d_ex00"][:, :, :], ex[:])
                        for i in range(2):
                            kc = kcp * 2 + i
                            # rows 0-63: attn; row 64: denominator (ones col)
                            nc.tensor.matmul(
                                av[ds(0, 65), :],
                                v_sb[:, kc, ds(h * 65, 65)], ex[:, i, :],
                                start=(kc == 0), stop=(kc == NKC - 1))
                    rec = nrmp.tile([1, 512], F32, tag="rec")
                    nc.vector.reciprocal(rec[:], av[ds(64, 1), :])
                    if dbg and qb == 0 and h == 0:
                        nc.sync.dma_start(dbg["d_den00"][:, :], rec[:])
                    rbc = nrmp.tile([128, 512], F32, tag="rbc",
                                    name=f"rbc{it}_{ra}_{qb}_{h}")
                    nc.gpsimd.partition_broadcast(rbc[:, :], rec[:])
                    if dbg and qb == 0 and h == 0:
                        nc.sync.dma_start(dbg["d_rbc00"][:, :], rbc[:])
                    if h % 2 == 0:
                        nc.vector.tensor_tensor(
                            attn_nT[ds(0, 64), hc, qsl], av[ds(0, 64), :],
                            rbc[ds(0, 64), :], op=OP.mult)
                    else:
                        nc.vector.tensor_tensor(
                            stg[:, hc, qsl], av[ds(0, 64), :],
                            rbc[ds(0, 64), :], op=OP.mult)
                # odd-head partition shift (DVE cannot write across partitions)
                nc.sync.dma_start(attn_nT[ds(64, 64), :, qsl], stg[:, :, qsl])
                # out projection for this query block
                for tc_ in range(4):
                    tt = qb * 4 + tc_
                    o_st = outp.tile([128, D], F32, tag="ost")
                    for nh in range(2):
                        po2 = scps.tile([128, 2, 512], F32, tag="sc",
                                        name=f"po{it}_{ra}_{tt}_{nh}")
                        po = po2[:, 0, :]
                        for jc in range(2):
                            nc.tensor.matmul(
                                po, attn_nT[:, jc, ds(tt * 128, 128)],
                                wo[:, jc, ds(nh * 512, 512)],
                                start=(jc == 0), stop=(jc == 1))
                        nc.vector.tensor_scalar(
                            o_st[:, ds(nh * 512, 512)], po, 1.0, None,
                            op0=OP.mult)
                    nc.sync.dma_start(out_ext[ds(tt * 128, 128), :], o_st[:])
        if dbg:
            nc.sync.dma_start(dbg["d_qk"][:, :, :], qk_sb[:])
            nc.sync.dma_start(dbg["d_kpad"][:, :, :], k_pad[:])
            nc.sync.dma_start(dbg["d_v"][:, :, :], v_sb[:])
            nc.sync.dma_start(dbg["d_attnT"][:, :, :], attn_nT[:])
            nc.sync.dma_start(dbg["d_abc"][:, :], a_bc[:])
            nc.sync.dma_start(dbg["d_cbc"][:, :], c_bc[:])
            nc.sync.dma_start(dbg["d_atok"][:, :], a_tok[:])


def build_bass(n_iters=1):
    nc = bacc.Bacc(None, num_devices=N_CORES)
    xT_ext = nc.declare_dram_parameter("xT", [D, S], BF16, isOutput=False)
    wq_ext = nc.declare_dram_parameter("w_qkv", [D, 768], BF16, isOutput=False)
    wsum_ext = nc.declare_dram_parameter("qkv_wsum", [768], F32, isOutput=False)
    wo_ext = nc.declare_dram_parameter("w_out", [256, D], BF16, isOutput=False)
    out_ext = nc.declare_dram_parameter("out", [S, D], F32, isOutput=True)
    ext = (xT_ext, wq_ext, wsum_ext, wo_ext, out_ext)
    with tile.TileContext(nc) as tc:
        for it in range(n_iters):
            _build_iter(nc, tc, ext, it)
    nc.finalize()
    return nc


def make_in_maps(x, ln_scale, ln_bias, w_qkv, w_out, b_out):
    bf = ml_dtypes.bfloat16
    lns = np.asarray(ln_scale, np.float32)
    lnb = np.asarray(ln_bias, np.float32)
    wq_f = np.asarray(w_qkv, np.float32) * lns[:, None]   # fold ln scale
    # ln_bias contributes lnb @ w_qkv, a constant row — zero for this model
    assert np.abs(lnb @ np.asarray(w_qkv, np.float32)).max() < 1e-6, \
        "nonzero ln_bias not supported by this kernel"
    xTbf = [np.ascontiguousarray(np.asarray(x[b], np.float32).T).astype(bf)
            for b in range(B)]
    in_maps = []
    for core in range(N_CORES):
        b, hg = core // H_PC, core % H_PC
        cols = slice(hg * 256, (hg + 1) * 256)
        wq_slice = np.concatenate(
            [wq_f[:, 0:INNER][:, cols], wq_f[:, INNER:2 * INNER][:, cols],
             wq_f[:, 2 * INNER:3 * INNER][:, cols]], axis=1)  # [1024, 768]
        wsum = wq_slice.sum(0).astype(np.float32)
        wo_slice = np.ascontiguousarray(
            np.asarray(w_out, np.float32)[hg * 256:(hg + 1) * 256, :]).astype(bf)
        in_maps.append({
            "xT": xTbf[b],
            "w_qkv": np.ascontiguousarray(wq_slice).astype(bf),
            "qkv_wsum": wsum,
            "w_out": wo_slice,
        })
    return in_maps


_CACHED_NC = None


def kernel(x, ln_scale, ln_bias, w_qkv, w_out, b_out):
    global _CACHED_NC
    if _CACHED_NC is None:
        _CACHED_NC = build_bass(n_iters=1)
    in_maps = make_in_maps(x, ln_scale, ln_bias, w_qkv, w_out, b_out)
    res = run_bass_kernel_spmd(_CACHED_NC, in_maps, list(range(N_CORES)))
    out = np.zeros((B, S, D), np.float32)
    for core in range(N_CORES):
        b = core // H_PC
        out[b] += res.results[core]["out"]
    out += np.asarray(b_out, np.float32)[None, None, :]
    return out


# revision 28
# speedup vs baseline: 1.2278x; 1.2278x over previous
"""Distributed Bass kernel for pre-LN multi-head attention on 8 TRN2 NeuronCores.

Problem: x[2, 2048, 1024] -> LayerNorm -> QKV (16 heads x 64) -> softmax(QK^T/8)V
         -> out proj [1024] + bias.

Sharding (v2): core = (batch b, head group hg) — data parallel over B, tensor
parallel over heads (4 heads/core). Each core projects Q/K/V only for its own
4 heads over the full 2048 tokens (no redundant K/V recompute), runs attention,
and emits a PARTIAL output projection [2048, 1024] (f32). The host sums the 4
partials per batch and adds b_out — the same class of work as gather/unshard.

Measured-HW-calibrated choices (see probes.py):
- 64-contraction score matmuls run ~3x slower per row than 128-contraction on
  real HW, so K is stored zero-padded to 128 rows per head (parity trick: even
  heads real in rows 0-63, odd heads in 64-127; the zero rows null out the
  other head's Q in the full-128-partition moving operand).
- PE transposes are ~4x the model cost; attention output is produced directly
  transposed ([dh, q] orientation) so none are needed.
- exp runs ~2x faster than the model (≈0.46 ns/elem) — the Act engine only
  does exp (LN stats are ones-matmuls on PE). The softmax denominator rides
  the av matmul as a 65th ones-column of V (separate [128,1]-stationary den
  matmuls measured 2.2x slower for the whole attention loop — they thrash the
  PE weight pipeline). All heads' av lands at partitions 0-64; odd heads are
  staged and partition-shifted to rows 64-127 with one SBUF->SBUF DMA per
  query block (DVE cannot write across partitions).
- LayerNorm is folded into the projections as a rank-1 correction:
  proj[col,t] = a[t]*(W^T x)[col,t] + c[t]*colsum(W)[col], a=rstd, c=-mean*rstd
  (ln_scale folded into W host-side; ln_bias@W == 0 for this model). This lets
  raw QKV matmuls start before LN stats finish.

All matmul operands bf16 (fp8 fails the 2e-2 max-rel-err gate — measured),
accumulation f32.
"""

import numpy as np
import ml_dtypes

import concourse.bass as bass
import concourse.mybir as mybir
import concourse.tile as tile
from concourse import bacc
from concourse.bass import ts, ds
from concourse.bass_utils import run_bass_kernel_spmd

B, S, D = 2, 2048, 1024
H, DH = 16, 64
INNER = H * DH
N_CORES = 8
H_PC = 4               # heads per core
NDC = 8                # 128-row contraction chunks over D
NTB = 4                # token blocks of 512
NKC = 16               # kpos chunks of 128
NTT = 16               # token tiles of 128
F32 = mybir.dt.float32
BF16 = mybir.dt.bfloat16
AF = mybir.ActivationFunctionType
OP = mybir.AluOpType


DEBUG = False


def _build_iter(nc, tc, ext, it, rep_proj=1, rep_attn=1):
    (xT_ext, wq_ext, wsum_ext, wo_ext, out_ext) = ext
    dbg = {}
    if DEBUG and it == 0:
        for nm, shp, dt in [("d_qk", [128, 4, S], BF16),
                            ("d_kpad", [128, H_PC, S], BF16),
                            ("d_v", [128, NKC, H_PC * 65], BF16),
                            ("d_attnT", [128, 2, S], BF16),
                            ("d_abc", [128, S], BF16),
                            ("d_cbc", [128, S], BF16),
                            ("d_atok", [128, NKC], F32),
                            ("d_ex00", [128, 2, 512], BF16),
                            ("d_den00", [1, 512], F32),
                            ("d_rbc00", [128, 512], F32)]:
            dbg[nm] = nc.declare_dram_parameter(nm, shp, dt, isOutput=True)

    with tc.tile_pool(name=f"const{it}", bufs=1) as constp, \
         tc.tile_pool(name=f"pers{it}", bufs=1) as pers, \
         tc.tile_pool(name=f"dram{it}", bufs=1, space="DRAM") as dram:

        # ---- constants ----
        ones_st = constp.tile([128, 1], BF16)
        nc.vector.memset(ones_st[:], 1.0)
        eps_t = constp.tile([1, 1], F32)
        nc.vector.memset(eps_t[:], 1e-6)
        wsum_t = constp.tile([128, 6], F32)       # per-partition col sums
        nc.sync.dma_start(wsum_t[:], wsum_ext[:].rearrange("(c p) -> p c", p=128))
        vwsum_bc = constp.tile([128, 256], F32)   # v col sums, bcast across parts
        nc.sync.dma_start(
            vwsum_bc[:],
            wsum_ext[ds(512, 256)].rearrange(
                "(o d) -> o d", o=1)[0:1, :].to_broadcast((128, 256)))

        # ---- persistent activations ----
        xT = pers.tile([128, NDC, S], BF16)       # raw x^T (d = c*128+p)
        qk_sb = pers.tile([128, 4, S], BF16)      # q (cc 0-1), k packed (cc 2-3)
        k_pad = pers.tile([128, H_PC, S], BF16)   # per-head K, parity-zero-padded
        v_sb = pers.tile([128, NKC, H_PC * 65], BF16)  # V + ones col per head
        stg = pers.tile([64, 2, S], BF16)         # odd-head attn staging
        attn_nT = pers.tile([128, 2, S], BF16)    # normalized attn out, [inner, q]
        a_bc = pers.tile([128, S], BF16)          # rstd, bcast across partitions
        c_bc = pers.tile([128, S], BF16)          # -mean*rstd, bcast
        a_tok = pers.tile([128, NKC], F32)        # rstd, tokens on partitions
        c_tok = pers.tile([128, NKC], F32)

        a_dram = dram.tile([S], BF16)
        c_dram = dram.tile([S], BF16)
        a32_dram = dram.tile([S], F32)
        c32_dram = dram.tile([S], F32)

        wq = pers.tile([128, NDC, 768], BF16)
        wo = pers.tile([128, 2, D], BF16)

        # ones cols of v_sb (softmax denominator rides the av matmul)
        nc.gpsimd.memset(
            v_sb[:].rearrange("p k (h c) -> p k h c", c=65)[:, :, :, 64:65], 1.0)
        # zero halves of k_pad (parity trick)
        for h in range(H_PC):
            nc.gpsimd.memset(k_pad[ds((1 - h % 2) * 64, 64), h, :], 0.0)

        # ---- loads: weights first (they gate every proj matmul; the xT
        # bulk would otherwise serialize ahead of them on the DMA queue) ----
        nc.sync.dma_start(wq[:], wq_ext[:, :].rearrange("(c p) n -> p c n", p=128))
        for tb in range(NTB):
            nc.sync.dma_start(
                xT[:, :, ds(tb * 512, 512)],
                xT_ext[:, ds(tb * 512, 512)].rearrange("(c p) t -> p c t", p=128))
        nc.sync.dma_start(wo[:], wo_ext[:, :].rearrange("(c p) n -> p c n", p=128))

        # =============== LN stats (PE ones-matmuls) -> a, c ===============
        with tc.tile_pool(name=f"st{it}", bufs=2) as stp, \
             tc.tile_pool(name=f"stps{it}", bufs=3, space="PSUM") as stps:
            sq = pers.tile([128, NDC, S], BF16)
            for tb in range(NTB):
                sl = ds(tb * 512, 512)
                nc.vector.tensor_tensor(
                    sq[:, :, sl], xT[:, :, sl], xT[:, :, sl], op=OP.mult)
                s_ps = stps.tile([1, 512], F32, tag="s")
                q_ps = stps.tile([1, 512], F32, tag="q")
                for dc in range(NDC):
                    nc.tensor.matmul(s_ps[:], ones_st[:], xT[:, dc, sl],
                                     start=(dc == 0), stop=(dc == NDC - 1))
                for dc in range(NDC):
                    nc.tensor.matmul(q_ps[:], ones_st[:], sq[:, dc, sl],
                                     start=(dc == 0), stop=(dc == NDC - 1))
                mean = stp.tile([1, 512], F32, tag="mean")
                nc.vector.tensor_scalar(mean[:], s_ps[:], 1.0 / D, None, op0=OP.mult)
                msq = stp.tile([1, 512], F32, tag="msq")
                nc.vector.tensor_tensor(msq[:], mean[:], mean[:], op=OP.mult)
                var = stp.tile([1, 512], F32, tag="var")
                nc.vector.scalar_tensor_tensor(
                    var[:], q_ps[:], 1.0 / D, msq[:], op0=OP.mult, op1=OP.subtract)
                std = stp.tile([1, 512], F32, tag="std")
                nc.scalar.activation(std[:], var[:], AF.Sqrt, bias=eps_t[:])
                rstd = stp.tile([1, 512], F32, tag="rstd")
                nc.vector.reciprocal(rstd[:], std[:])
                c_f = stp.tile([1, 512], F32, tag="c_f")
                nc.vector.scalar_tensor_tensor(
                    c_f[:], mean[:], -1.0, rstd[:], op0=OP.mult, op1=OP.mult)
                a_bf = stp.tile([1, 512], BF16, tag="a_bf")
                nc.vector.tensor_scalar(a_bf[:], rstd[:], 1.0, None, op0=OP.mult)
                c_bf = stp.tile([1, 512], BF16, tag="c_bf")
                nc.vector.tensor_scalar(c_bf[:], c_f[:], 1.0, None, op0=OP.mult)
                sl1 = ds(tb * 512, 512)
                nc.sync.dma_start(
                    a_dram[sl1].rearrange("(o t) -> o t", o=1), a_bf[:])
                nc.sync.dma_start(
                    c_dram[sl1].rearrange("(o t) -> o t", o=1), c_bf[:])
                nc.sync.dma_start(
                    a32_dram[sl1].rearrange("(o t) -> o t", o=1), rstd[:])
                nc.sync.dma_start(
                    c32_dram[sl1].rearrange("(o t) -> o t", o=1), c_f[:])
            nc.sync.dma_start(
                a_bc[:],
                a_dram[:].rearrange("(o t) -> o t", o=1)[0:1, :].to_broadcast((128, S)))
            nc.sync.dma_start(
                c_bc[:],
                c_dram[:].rearrange("(o t) -> o t", o=1)[0:1, :].to_broadcast((128, S)))
            nc.sync.dma_start(a_tok[:], a32_dram[:].rearrange("(k p) -> p k", p=128))
            nc.sync.dma_start(c_tok[:], c32_dram[:].rearrange("(k p) -> p k", p=128))

        # =============== QKV projections (raw x; LN as rank-1 fixup) =========
        for rp in range(rep_proj):
          with tc.tile_pool(name=f"qv{it}_{rp}", bufs=3) as qvp, \
             tc.tile_pool(name=f"qkps{it}_{rp}", bufs=3, space="PSUM") as qkps:
            # K first so scores can start earliest; cc: 0-1 q, 2-3 k
            for cc in (2, 3, 0, 1):
                for tb in range(NTB):
                    sl = ds(tb * 512, 512)
                    ps = qkps.tile([128, 512], F32, tag="qk")
                    for dc in range(NDC):
                        nc.tensor.matmul(ps[:], wq[:, dc, ts(cc, 128)],
                                         xT[:, dc, sl],
                                         start=(dc == 0), stop=(dc == NDC - 1))
                    t1 = qvp.tile([128, 512], BF16, tag="t1")
                    nc.vector.tensor_tensor(t1[:], ps[:], a_bc[:, sl], op=OP.mult)
                    if cc < 2:
                        nc.vector.scalar_tensor_tensor(
                            qk_sb[:, cc, sl], c_bc[:, sl], wsum_t[:, cc:cc + 1],
                            t1[:], op0=OP.mult, op1=OP.add)
                    else:
                        for par in range(2):       # head = (cc-2)*2 + par
                            h = (cc - 2) * 2 + par
                            pr = ds(par * 64, 64)
                            nc.vector.scalar_tensor_tensor(
                                k_pad[pr, h, sl], c_bc[pr, sl],
                                wsum_t[pr, cc:cc + 1], t1[pr, :],
                                op0=OP.mult, op1=OP.add)
            # V in [token, vcol] orientation
            for kc in range(NKC):
                ps = qkps.tile([128, 256], F32, tag="v")
                for dc in range(NDC):
                    nc.tensor.matmul(ps[:], xT[:, dc, ds(kc * 128, 128)],
                                     wq[:, dc, ds(512, 256)],
                                     start=(dc == 0), stop=(dc == NDC - 1))
                t1 = qvp.tile([128, 256], BF16, tag="vt1")
                nc.scalar.activation(t1[:], ps[:], AF.Copy,
                                     scale=a_tok[:, kc:kc + 1])
                nc.vector.scalar_tensor_tensor(
                    v_sb[:, kc, :].rearrange("p (h c) -> p h c", c=65)[:, :, 0:64],
                    vwsum_bc[:].rearrange("p (h c) -> p h c", c=64),
                    c_tok[:, kc:kc + 1],
                    t1[:].rearrange("p (h c) -> p h c", c=64),
                    op0=OP.mult, op1=OP.add)

        # =============== attention + output projection ===============
        for ra in range(rep_attn):
          with tc.tile_pool(name=f"att{it}_{ra}", bufs=5) as attp, \
             tc.tile_pool(name=f"nrm{it}_{ra}", bufs=4) as nrmp, \
             tc.tile_pool(name=f"out{it}_{ra}", bufs=2) as outp, \
             tc.tile_pool(name=f"scps{it}_{ra}", bufs=3, space="PSUM") as scps, \
             tc.tile_pool(name=f"avps{it}_{ra}", bufs=2, space="PSUM") as avps:
            for qb in range(NTB):
                qsl = ds(qb * 512, 512)
                for h in range(H_PC):
                    hc = h // 2
                    q_mv = qk_sb[:, hc, qsl]
                    av = avps.tile([128, 512], F32, tag="av",
                                   name=f"av{it}_{ra}_{qb}_{h}")
                    for kcp in range(8):
                        sc = scps.tile([128, 2, 512], F32, tag="sc",
                                       name=f"sc{it}_{ra}_{qb}_{h}_{kcp}")
                        for i in range(2):
                            kc = kcp * 2 + i
                            nc.tensor.matmul(
                                sc[:, i, :], k_pad[:, h, ds(kc * 128, 128)], q_mv,
                                start=True, stop=True)
                        ex = attp.tile([128, 2, 512], BF16, tag="ex")
                        nc.scalar.activation(ex[:], sc[:], AF.Exp, scale=0.125)
                        if dbg and qb == 0 and h == 0 and kcp == 0:
                            nc.sync.dma_start(dbg["d_ex00"][:, :, :], ex[:])
                        for i in range(2):
                            kc = kcp * 2 + i
                            # rows 0-63: attn; row 64: denominator (ones col)
                            nc.tensor.matmul(
                                av[ds(0, 65), :],
                                v_sb[:, kc, ds(h * 65, 65)], ex[:, i, :],
                                start=(kc == 0), stop=(kc == NKC - 1))
                    rec = nrmp.tile([1, 512], F32, tag="rec")
                    nc.vector.reciprocal(rec[:], av[ds(64, 1), :])
                    if dbg and qb == 0 and h == 0:
                        nc.sync.dma_start(dbg["d_den00"][:, :], rec[:])
                    rbc = nrmp.tile([128, 512], F32, tag="rbc",
                                    name=f"rbc{it}_{ra}_{qb}_{h}")
                    nc.gpsimd.partition_broadcast(rbc[:, :], rec[:])
                    if dbg and qb == 0 and h == 0:
                        nc.sync.dma_start(dbg["d_rbc00"][:, :], rbc[:])
                    if h % 2 == 0:
                        nc.vector.tensor_tensor(
                            attn_nT[ds(0, 64), hc, qsl], av[ds(0, 64), :],
                            rbc[ds(0, 64), :], op=OP.mult)
                    else:
                        nc.vector.tensor_tensor(
                            stg[:, hc, qsl], av[ds(0, 64), :],
                            rbc[ds(0, 64), :], op=OP.mult)
                # odd-head partition shift (DVE cannot write across partitions)
                nc.sync.dma_start(attn_nT[ds(64, 64), :, qsl], stg[:, :, qsl])
                # out projection for this query block
                for tc_ in range(4):
                    tt = qb * 4 + tc_
                    o_st = outp.tile([128, D], F32, tag="ost")
                    for nh in range(2):
                        po2 = scps.tile([128, 2, 512], F32, tag="sc",
                                        name=f"po{it}_{ra}_{tt}_{nh}")
                        po = po2[:, 0, :]
                        for jc in range(2):
                            nc.tensor.matmul(
                                po, attn_nT[:, jc, ds(tt * 128, 128)],
                                wo[:, jc, ds(nh * 512, 512)],
                                start=(jc == 0), stop=(jc == 1))
                        nc.vector.tensor_scalar(
                            o_st[:, ds(nh * 512, 512)], po, 1.0, None,
                            op0=OP.mult)
                    nc.sync.dma_start(out_ext[ds(tt * 128, 128), :], o_st[:])
        if dbg:
            nc.sync.dma_start(dbg["d_qk"][:, :, :], qk_sb[:])
            nc.sync.dma_start(dbg["d_kpad"][:, :, :], k_pad[:])
            nc.sync.dma_start(dbg["d_v"][:, :, :], v_sb[:])
            nc.sync.dma_start(dbg["d_attnT"][:, :, :], attn_nT[:])
            nc.sync.dma_start(dbg["d_abc"][:, :], a_bc[:])
            nc.sync.dma_start(dbg["d_cbc"][:, :], c_bc[:])
            nc.sync.dma_start(dbg["d_atok"][:, :], a_tok[:])


def build_bass(n_iters=1):
    nc = bacc.Bacc(None, num_devices=N_CORES)
    xT_ext = nc.declare_dram_parameter("xT", [D, S], BF16, isOutput=False)
    wq_ext = nc.declare_dram_parameter("w_qkv", [D, 768], BF16, isOutput=False)
    wsum_ext = nc.declare_dram_parameter("qkv_wsum", [768], F32, isOutput=False)
    wo_ext = nc.declare_dram_parameter("w_out", [256, D], BF16, isOutput=False)
    out_ext = nc.declare_dram_parameter("out", [S, D], F32, isOutput=True)
    ext = (xT_ext, wq_ext, wsum_ext, wo_ext, out_ext)
    with tile.TileContext(nc) as tc:
        for it in range(n_iters):
            _build_iter(nc, tc, ext, it)
    nc.finalize()
    return nc


def make_in_maps(x, ln_scale, ln_bias, w_qkv, w_out, b_out):
    bf = ml_dtypes.bfloat16
    lns = np.asarray(ln_scale, np.float32)
    lnb = np.asarray(ln_bias, np.float32)
    wq_f = np.asarray(w_qkv, np.float32) * lns[:, None]   # fold ln scale
    # ln_bias contributes lnb @ w_qkv, a constant row — zero for this model
    assert np.abs(lnb @ np.asarray(w_qkv, np.float32)).max() < 1e-6, \
        "nonzero ln_bias not supported by this kernel"
    xTbf = [np.ascontiguousarray(np.asarray(x[b], np.float32).T).astype(bf)
            for b in range(B)]
    in_maps = []
    for core in range(N_CORES):
        b, hg = core // H_PC, core % H_PC
        cols = slice(hg * 256, (hg + 1) * 256)
        wq_slice = np.concatenate(
            [wq_f[:, 0:INNER][:, cols], wq_f[:, INNER:2 * INNER][:, cols],
             wq_f[:, 2 * INNER:3 * INNER][:, cols]], axis=1)  # [1024, 768]
        wsum = wq_slice.sum(0).astype(np.float32)
        wo_slice = np.ascontiguousarray(
            np.asarray(w_out, np.float32)[hg * 256:(hg + 1) * 256, :]).astype(bf)
        in_maps.append({
            "xT": xTbf[b],
            "w_qkv": np.ascontiguousarray(wq_slice).astype(bf),
            "qkv_wsum": wsum,
            "w_out": wo_slice,
        })
    return in_maps


_CACHED_NC = None


def kernel(x, ln_scale, ln_bias, w_qkv, w_out, b_out):
    global _CACHED_NC
    if _CACHED_NC is None:
        _CACHED_NC = build_bass(n_iters=1)
    in_maps = make_in_maps(x, ln_scale, ln_bias, w_qkv, w_out, b_out)
    res = run_bass_kernel_spmd(_CACHED_NC, in_maps, list(range(N_CORES)))
    out = np.zeros((B, S, D), np.float32)
    for core in range(N_CORES):
        b = core // H_PC
        out[b] += res.results[core]["out"]
    out += np.asarray(b_out, np.float32)[None, None, :]
    return out
